# revision 6
# baseline (speedup 1.0000x reference)
"""Trainium2 Bass kernel for a multi-head attention block (B=2, S=2048, D=1024, H=16).

Returns (out, attention_scores) matching the reference:
    Q = q @ w_q + b_q ; K = k @ w_k + b_k ; V = v @ w_v + b_v   (per-head split)
    P = softmax(mask(QK^T / sqrt(dk)))          -> attention_scores [B,H,S,S]
    out = (P @ V merged) @ w_o + b_o            -> [B,S,D]

Sharding: 8 cores, each takes one (batch, 4-heads) shard: cores 0-3 -> batch 0
heads 4c..4c+3, cores 4-7 -> batch 1. Projection weights are column-sliced per
core; w_o is row-sliced and partial outputs are summed on the host.

Device algorithm per core (all fp32):
  phase 1: QT/KT [256,S] (head-dim on partitions) and V [S,256] projections on PE,
           biases fused via ACT Identity-with-bias epilogue / ones-row matmul.
  phase 2 (pass A): scores tile [128qu,S] = QT^T-slice @ KT + identity-matmul mask
           bias add; one ACT Exp with fused row-sum accumulate; DVE reciprocal +
           normalize; DMA P out. 1/sqrt(dk) is pre-folded into w_q/b_q (exact).
  phase 3 (pass B): scores recomputed transposed [128k, S/2 q] (+ transposed mask
           bias), ACT Exp, PE matmul with V accumulates unnormalized ctx^T in
           PSUM; normalized with a ones-matmul broadcast of the pass-A
           reciprocals (DRAM round trip).
  phase 4: out_partial [S, D] = ctxT^T @ w_o_rows on PE.
"""

import os
import sys

import numpy as np

if "/opt/trn_rl_repo" not in sys.path:
    sys.path.insert(0, "/opt/trn_rl_repo")

B, S_FULL, D, H = 2, 2048, 1024, 16
DK = 64
HLOC = 4  # heads per core
NCORES = 8
SCALE = 1.0 / 8.0  # 1/sqrt(DK), exact power of two
NEG = -1.0e30


def _split_waits(nc, cap=1):
    """Walrus codegen on this toolchain rejects >1 sync-wait per instruction
    (setupSyncWait: 'Too many sync wait commands'). Move excess waits onto
    inserted same-engine NoOps, which execute ahead of the instruction on its
    sequencer queue, preserving ordering semantics."""
    import concourse.mybir as mybir

    skip = {"Call", "UnconditionalBranch", "ConditionalBranch"}
    for blk in nc.m.functions[0].blocks:
        newinsts = []
        for ins in blk.instructions:
            si = ins.sync_info
            if (
                ins.opcode not in skip
                and si is not None
                and si.on_wait
                and len(si.on_wait) > cap
            ):
                move = list(si.on_wait[:-cap])
                si.on_wait = list(si.on_wait[-cap:])
                for i in range(0, len(move), cap):
                    nop = mybir.InstNoOp(name=f"{ins.name}-wn{i}", ins=[], outs=[])
                    nop.engine = ins.engine
                    nop.sync_info = mybir.SyncInfo(
                        on_wait=move[i:i + cap], on_update=[]
                    )
                    newinsts.append(nop)
            newinsts.append(ins)
        blk.instructions = newinsts


def build_module(S=S_FULL):
    import concourse.bass as bass
    import concourse.mybir as mybir
    import concourse.tile as tile
    from concourse.masks import make_identity

    fp32 = mybir.dt.float32
    AF = mybir.ActivationFunctionType

    QT_TILES = 2          # 256 local head dims -> 2 partition tiles
    KC = D // 128         # 8 contraction chunks for projections
    NQT = S // 128        # q tiles
    NK512 = S // 512      # 512-wide chunks of S
    S2 = S // 2           # pass-B q half width
    NKT = S // 128        # k tiles (pass B)
    N2 = max(S2 // 512, 1)
    C2 = min(S2, 512)     # pass-B free chunk

    nc = bass.Bass(trn_type="TRN2", debug=False)

    def din(name, shape):
        return nc.dram_tensor(name, shape, fp32, kind="ExternalInput").ap()

    xq_t = din("xq_t", [D, S])
    xk_t = din("xk_t", [D, S])
    xv_t = din("xv_t", [D, S])
    wq = din("wq", [D, 256])
    wk = din("wk", [D, 256])
    wv = din("wv", [D, 256])
    bq = din("bq", [128, 2])
    bk = din("bk", [128, 2])
    bv_row = din("bv_row", [1, 256])
    wo = din("wo", [256, D])
    maskbias = din("maskbias", [S, S])
    maskbias_t = din("maskbias_t", [S, S])

    p_out = nc.dram_tensor("p_out", [HLOC, S, S], fp32, kind="ExternalOutput").ap()
    o_part = nc.dram_tensor("o_part", [S, D], fp32, kind="ExternalOutput").ap()
    recips_d = nc.dram_tensor("recips_d", [HLOC, S], fp32, kind="Internal").ap()

    with tile.TileContext(nc) as tc:
        with tc.tile_pool(name="resident", bufs=1) as res:
            ident = res.tile([128, 128], fp32, name="ident")
            make_identity(nc, ident)
            ones = res.tile([1, 128], fp32, name="ones")
            nc.gpsimd.memset(ones, 1.0)
            bq_s = res.tile([128, 2], fp32, name="bq_s")
            nc.sync.dma_start(bq_s, bq)
            bk_s = res.tile([128, 2], fp32, name="bk_s")
            nc.sync.dma_start(bk_s, bk)
            bv_s = res.tile([1, 256], fp32, name="bv_s")
            nc.sync.dma_start(bv_s, bv_row)

            QT = [res.tile([128, S], fp32, name=f"QT{m}") for m in range(QT_TILES)]
            KT = [res.tile([128, S], fp32, name=f"KT{m}") for m in range(QT_TILES)]
            V_s = res.tile([128, S * 2], fp32, name="V_s")  # seq tile t at cols t*256
            ctxT = [res.tile([128, S], fp32, name=f"ctxT{p}") for p in range(2)]
            recip_all = res.tile([128, HLOC * NQT], fp32, name="recip_all")

            # ---------------- phase 1: projections ----------------
            with tc.tile_pool(name="wpool", bufs=1) as wp, \
                 tc.tile_pool(name="xstream", bufs=10) as xs, \
                 tc.tile_pool(name="pp1", bufs=4, space="PSUM") as pp1:
                wq_s = wp.tile([128, KC * 256], fp32, name="wq_s")
                wk_s = wp.tile([128, KC * 256], fp32, name="wk_s")
                wv_s = wp.tile([128, KC * 256], fp32, name="wv_s")
                for w_s, w_d in ((wq_s, wq), (wk_s, wk), (wv_s, wv)):
                    nc.sync.dma_start(
                        w_s.rearrange("p (c n) -> p c n", c=KC),
                        w_d.rearrange("(c p) n -> p c n", p=128),
                    )

                for x_d, w_s, b_s, outT in (
                    (xq_t, wq_s, bq_s, QT),
                    (xk_t, wk_s, bk_s, KT),
                ):
                    for n in range(NK512):
                        xts = []
                        for kc in range(KC):
                            xt = xs.tile([128, 512], fp32, name="xt", tag="xt")
                            nc.sync.dma_start(
                                xt, x_d[kc * 128:(kc + 1) * 128, n * 512:(n + 1) * 512]
                            )
                            xts.append(xt)
                        for m in range(QT_TILES):
                            ps = pp1.tile([128, 512], fp32, name="ps_p", tag="proj")
                            for kc in range(KC):
                                c0 = kc * 256 + m * 128
                                nc.tensor.matmul(
                                    ps, w_s[:, c0:c0 + 128], xts[kc],
                                    start=(kc == 0), stop=(kc == KC - 1),
                                )
                            nc.scalar.activation(
                                outT[m][:, n * 512:(n + 1) * 512], ps,
                                AF.Identity, bias=b_s[:, m:m + 1],
                            )
                # V: natural [seq, dlocal] layout
                for sg in range(NK512):
                    xts = []
                    for kc in range(KC):
                        xt = xs.tile([128, 512], fp32, name="xt", tag="xt")
                        nc.sync.dma_start(
                            xt, xv_t[kc * 128:(kc + 1) * 128, sg * 512:(sg + 1) * 512]
                        )
                        xts.append(xt)
                    for st in range(4):
                        tg = sg * 4 + st
                        ps = pp1.tile([128, 256], fp32, name="ps_v", tag="vproj")
                        for kc in range(KC):
                            nc.tensor.matmul(
                                ps, xts[kc][:, st * 128:(st + 1) * 128],
                                wv_s[:, kc * 256:(kc + 1) * 256],
                                start=(kc == 0), stop=False,
                            )
                        nc.tensor.matmul(
                            ps, ones[:, 0:128], bv_s, start=False, stop=True,
                        )
                        nc.scalar.copy(V_s[:, tg * 256:(tg + 1) * 256], ps)

            # ---------------- phase 2: pass A (P out + reciprocals) -----------
            with tc.tile_pool(name="mbp", bufs=2) as mbp, \
                 tc.tile_pool(name="ep", bufs=3) as ep, \
                 tc.tile_pool(name="rp", bufs=4) as rp, \
                 tc.tile_pool(name="pp2", bufs=2, space="PSUM") as pp2:
                for qt in range(NQT):
                    mb = mbp.tile([128, S], fp32, name="mb", tag="mb")
                    nc.sync.dma_start(mb, maskbias[qt * 128:(qt + 1) * 128, :])
                    for h in range(HLOC):
                        m, po = h // 2, (h % 2) * 64
                        ps = pp2.tile([128, S], fp32, name="ps_s", tag="scores")
                        for nk in range(NK512):
                            ch = slice(nk * 512, (nk + 1) * 512)
                            nc.tensor.matmul(
                                ps[:, ch],
                                QT[m][po:po + 64, qt * 128:(qt + 1) * 128],
                                KT[m][po:po + 64, ch],
                                start=True, stop=False,
                            )
                            nc.tensor.matmul(
                                ps[:, ch], ident, mb[:, ch],
                                start=False, stop=True,
                            )
                        e = ep.tile([128, S], fp32, name="e", tag="e")
                        rs = rp.tile([128, 1], fp32, name="rs", tag="rs")
                        nc.scalar.activation(e, ps, AF.Exp, accum_out=rs)
                        idx = h * NQT + qt
                        rc = recip_all[:, idx:idx + 1]
                        nc.vector.reciprocal(rc, rs)
                        nc.vector.tensor_scalar_mul(e, e, rc)
                        nc.sync.dma_start(p_out[h, qt * 128:(qt + 1) * 128, :], e)
                nc.sync.dma_start(
                    recips_d.rearrange("h (t p) -> p (h t)", p=128), recip_all
                )

            # ---------------- phase 3: pass B (ctxT) ----------------
            with tc.tile_pool(name="mtp", bufs=2) as mtp, \
                 tc.tile_pool(name="etp", bufs=3) as etp, \
                 tc.tile_pool(name="rrp", bufs=1) as rrp, \
                 tc.tile_pool(name="rbs", bufs=2) as rbs, \
                 tc.tile_pool(name="psb", bufs=2, space="PSUM") as psb, \
                 tc.tile_pool(name="ctxp", bufs=1, space="PSUM") as ctxp:
                rrow = []
                for h in range(HLOC):
                    rr = rrp.tile([1, S], fp32, name=f"rr{h}", tag=f"rr{h}")
                    nc.sync.dma_start(rr, recips_d[h:h + 1, :])
                    rrow.append(rr)
                for qh in range(2):
                    ctx_ps = [
                        ctxp.tile([128, S2], fp32, name=f"ctxps{p}", tag=f"ctxps{p}")
                        for p in range(2)
                    ]
                    for kt in range(NKT):
                        mt = mtp.tile([128, S2], fp32, name="mt", tag="mt")
                        nc.sync.dma_start(
                            mt,
                            maskbias_t[kt * 128:(kt + 1) * 128, qh * S2:(qh + 1) * S2],
                        )
                        for h in range(HLOC):
                            m, po = h // 2, (h % 2) * 64
                            ps = psb.tile([128, S2], fp32, name="ps_t", tag="sT")
                            for nq in range(N2):
                                ch = slice(nq * C2, (nq + 1) * C2)
                                gch = slice(qh * S2 + nq * C2, qh * S2 + (nq + 1) * C2)
                                nc.tensor.matmul(
                                    ps[:, ch],
                                    KT[m][po:po + 64, kt * 128:(kt + 1) * 128],
                                    QT[m][po:po + 64, gch],
                                    start=True, stop=False,
                                )
                                nc.tensor.matmul(
                                    ps[:, ch], ident, mt[:, ch],
                                    start=False, stop=True,
                                )
                            eT = etp.tile([128, S2], fp32, name="eT", tag="eT")
                            nc.scalar.activation(eT, ps, AF.Exp)
                            vcol = kt * 256 + h * 64
                            for nq in range(N2):
                                ch = slice(nq * C2, (nq + 1) * C2)
                                nc.tensor.matmul(
                                    ctx_ps[h // 2][po:po + 64, ch],
                                    V_s[:, vcol:vcol + 64],
                                    eT[:, ch],
                                    start=(kt == 0), stop=(kt == NKT - 1),
                                    tile_position=(0, po),
                                    # two heads share a bank at disjoint
                                    # partition ranges; sim's zero-region
                                    # tracker can't see the partition split
                                    skip_group_check=True,
                                )
                    for pair in range(2):
                        rb_ps = psb.tile([128, S2], fp32, name="rb_ps", tag="sT")
                        for sub in range(2):
                            h, po = pair * 2 + sub, sub * 64
                            for nq in range(N2):
                                ch = slice(nq * C2, (nq + 1) * C2)
                                nc.tensor.matmul(
                                    rb_ps[po:po + 64, ch],
                                    ones[:, 0:64],
                                    rrow[h][0:1, qh * S2 + nq * C2:qh * S2 + (nq + 1) * C2],
                                    start=True, stop=True,
                                    tile_position=(0, po),
                                )
                        rb_s = rbs.tile([128, S2], fp32, name="rb_s", tag="rb_s")
                        nc.scalar.copy(rb_s, rb_ps)
                        nc.vector.tensor_mul(
                            ctxT[pair][:, qh * S2:(qh + 1) * S2], ctx_ps[pair], rb_s
                        )

            # ---------------- phase 4: output projection ----------------
            with tc.tile_pool(name="wop", bufs=1) as wop, \
                 tc.tile_pool(name="op", bufs=3) as op, \
                 tc.tile_pool(name="pp4", bufs=3, space="PSUM") as pp4:
                wo_s = []
                for pair in range(2):
                    w = wop.tile([128, D], fp32, name=f"wo_s{pair}", tag=f"wo{pair}")
                    nc.sync.dma_start(w, wo[pair * 128:(pair + 1) * 128, :])
                    wo_s.append(w)
                for qt in range(NQT):
                    ps = pp4.tile([128, D], fp32, name="ps_o", tag="o")
                    for pair in range(2):
                        for nn in range(D // 512):
                            ch = slice(nn * 512, (nn + 1) * 512)
                            nc.tensor.matmul(
                                ps[:, ch],
                                ctxT[pair][:, qt * 128:(qt + 1) * 128],
                                wo_s[pair][:, ch],
                                start=(pair == 0), stop=(pair == 1),
                            )
                    ot = op.tile([128, D], fp32, name="ot", tag="ot")
                    nc.scalar.copy(ot, ps)
                    nc.sync.dma_start(o_part[qt * 128:(qt + 1) * 128, :], ot)

    return nc


def make_in_maps(q, k, v, mask, w_q, b_q, w_k, b_k, w_v, b_v, w_o, S=S_FULL):
    f32 = np.float32
    mask2d = np.asarray(mask).reshape(S, S)
    mb = np.where(mask2d == 0, f32(NEG), f32(0.0)).astype(f32)
    mbT = np.ascontiguousarray(mb.T)
    in_maps = []
    for c in range(NCORES):
        b = c // (NCORES // B)
        h0 = (c % (NCORES // B)) * HLOC
        sl = slice(h0 * DK, h0 * DK + HLOC * DK)
        in_maps.append({
            "xq_t": np.ascontiguousarray(np.asarray(q[b], f32).T),
            "xk_t": np.ascontiguousarray(np.asarray(k[b], f32).T),
            "xv_t": np.ascontiguousarray(np.asarray(v[b], f32).T),
            "wq": np.ascontiguousarray(np.asarray(w_q[:, sl], f32) * f32(SCALE)),
            "wk": np.ascontiguousarray(np.asarray(w_k[:, sl], f32)),
            "wv": np.ascontiguousarray(np.asarray(w_v[:, sl], f32)),
            "bq": np.ascontiguousarray(
                (np.asarray(b_q[sl], f32) * f32(SCALE)).reshape(2, 128).T
            ),
            "bk": np.ascontiguousarray(np.asarray(b_k[sl], f32).reshape(2, 128).T),
            "bv_row": np.ascontiguousarray(np.asarray(b_v[sl], f32).reshape(1, 256)),
            "wo": np.ascontiguousarray(np.asarray(w_o[sl, :], f32)),
            "maskbias": mb,
            "maskbias_t": mbT,
        })
    return in_maps


_NC = None


def kernel(q, k, v, mask, w_q, b_q, w_k, b_k, w_v, b_v, w_o, b_o):
    global _NC
    from concourse.bass_utils import run_bass_kernel_spmd

    if _NC is None:
        _NC = build_module(S_FULL)
        _split_waits(_NC)

    in_maps = make_in_maps(q, k, v, mask, w_q, b_q, w_k, b_k, w_v, b_v, w_o)
    trace = bool(int(os.environ.get("KERNEL_TRACE", "0")))
    res = run_bass_kernel_spmd(
        _NC, in_maps, core_ids=list(range(NCORES)), trace=trace
    )
    kernel.last_results = res

    probs = np.empty((B, H, S_FULL, S_FULL), np.float32)
    out = np.zeros((B, S_FULL, D), np.float64)
    for c in range(NCORES):
        b = c // (NCORES // B)
        h0 = (c % (NCORES // B)) * HLOC
        r = res.results[c]
        probs[b, h0:h0 + HLOC] = r["p_out"]
        out[b] += r["o_part"]
    out = (out + np.asarray(b_o, np.float64)).astype(np.float32)
    return out, probs
